# revision 30
# baseline (speedup 1.0000x reference)
"""Positional embedding lookup kernel for Trainium2 (8 NeuronCores).

Problem: out[b, t, :] = tok_weight[x[b, t], :] + pos_weight[t, :]
  x:          [4, 4096]  int32/int64 token ids in [0, 32000)
  tok_weight: [32000, 512] f32
  pos_weight: [4096, 512]  f32
  out:        [4, 4096, 512] f32

Sharding: split the 4096 positions into 8 contiguous chunks of 512; core c
handles positions [c*512, (c+1)*512) for ALL 4 batches (2048 tokens).  This
makes each core read only its 1MB slice of pos_weight (reused across the 4
batches) instead of a per-token 4MB read.

Per-core flat token order: i = 0..2047 walks (b, q) = (i//512, i%512),
i.e. flat_idx = x[:, c*512:(c+1)*512].ravel().  The gather lands token i at
SBUF partition i%128, column-block i//128, so column block col corresponds
to batch col//4, position sub-block col%4 — which aligns a whole batch's
512 tokens with the (identically laid out) pos tile for a single wide add.

The row gather uses the GPSIMD dma_gather custom op (one descriptor per
row, ~0.34ns/descriptor generation) in 4 chunks of 512 rows so gather,
add, and store pipeline; indices are int16 (vocab 32000 < 32768), packed
i -> [i%16, i//16] over 16 partitions and replicated across the 8 Q7 cores.
"""

import numpy as np

import concourse.bass as bass
import concourse.tile as tile
from concourse import library_config, mybir
from concourse.bass_utils import run_bass_kernel_spmd

B = 4
T = 4096
E = 512
VOCAB = 32000
N_CORES = 8
POS_PER_CORE = T // N_CORES          # 512
TOK_PER_CORE = B * POS_PER_CORE      # 2048
P = 128
N_TILES = TOK_PER_CORE // P          # 16 column blocks of 128 tokens
JQ = POS_PER_CORE // P               # 4 pos sub-blocks
CHUNKS = 4                           # one gather/add/store chunk per batch
TOK_PER_CHUNK = TOK_PER_CORE // CHUNKS   # 512
IDX_COLS = TOK_PER_CORE // 16        # 128 int16 idx columns

_CACHE = {}


def _split_multi_waits(nc: bass.Bass) -> None:
    """Walrus codegen allows one sync-wait slot per TPB instruction (the
    NEURON_ISA_TPB_EVENTS struct); Tile can emit several.  Move extra waits
    onto standalone NoOps on the same engine, just before the instruction."""
    for func in nc.m.functions:
        for blk in func.blocks:
            new_insts = []
            for inst in blk.instructions:
                si = inst.sync_info
                if si is not None and len(si.on_wait) > 1:
                    for w in si.on_wait[:-1]:
                        nop = mybir.InstNoOp(
                            name=nc.get_next_instruction_name(),
                            engine=inst.engine,
                            bass_nofuse=True,
                            sync_info=mybir.SyncInfo(on_wait=[w], on_update=[]),
                        )
                        nc.register_instruction(nop)
                        new_insts.append(nop)
                    inst.sync_info = mybir.SyncInfo(
                        on_wait=si.on_wait[-1:], on_update=si.on_update
                    )
                new_insts.append(inst)
            blk.instructions[:] = new_insts


def _build_program(
    reps: int = 1,
    outer: int = 1,
    variant: str = "full",
    nqueues: int = 1,
    single_packet: bool = True,
    chunks: int = CHUNKS,
    out_part_major: bool = False,
    store_alt: bool = False,
    bufs: int = 3,
) -> bass.Bass:
    """reps>1 unrolls the steady-state gather/add/store loop; outer>1 wraps
    it in a runtime For_i loop.  Used for timing: the wall-time delta
    between two total rep counts isolates device time.  variant isolates
    pipeline stages for benching: "full" | "gather" | "store"."""
    nc = bass.Bass(num_swdge_queues=nqueues)

    xti = nc.declare_dram_parameter(
        "xti", [P, IDX_COLS], mybir.dt.int16, isOutput=False
    )
    pos = nc.declare_dram_parameter(
        "pos", [POS_PER_CORE, E], mybir.dt.float32, isOutput=False
    )
    tok = nc.declare_dram_parameter(
        "tok", [VOCAB, E], mybir.dt.float32, isOutput=False
    )
    out_shape = [P, N_TILES, E] if out_part_major else [N_TILES, P, E]
    out = nc.declare_dram_parameter("out", out_shape, mybir.dt.float32, isOutput=True)

    with tile.TileContext(nc) as tc:
        with (
            tc.tile_pool(name="const", bufs=1) as const_pool,
            tc.tile_pool(name="work", bufs=bufs) as work_pool,
        ):
            # dma_gather lives in the 'mlp' GPSIMD firmware library
            nc.gpsimd.load_library(library_config.mlp)

            xti_t = const_pool.tile([P, IDX_COLS], mybir.dt.int16)
            nc.sync.dma_start(out=xti_t[:], in_=xti[:])

            # all 4 pos sub-blocks in one DMA: partition p, cols
            # [jq*E:(jq+1)*E] hold pos[jq*128 + p, :]
            pos_t = const_pool.tile([P, JQ * E], mybir.dt.float32)
            nc.sync.dma_start(
                out=pos_t[:].rearrange("p (jq e) -> p jq e", jq=JQ),
                in_=pos.rearrange("(jq p) e -> p jq e", p=P),
            )
            # tiny DVE op so the vector engine observes the const-load DMA
            # semaphores once; later adds then need only the gather wait.
            obs = const_pool.tile([P, 1], mybir.dt.float32, tag="obs")
            nc.vector.tensor_copy(out=obs[:], in_=pos_t[:, 0:1])

            assert chunks in (2, 4, 8, 16)
            ncols = N_TILES // chunks            # column blocks per chunk
            icols = IDX_COLS // chunks           # idx columns per chunk
            tok_per_chunk = TOK_PER_CORE // chunks
            nidx_reg = nc.gpsimd.to_reg(tok_per_chunk)

            def pos_in1(k):
                jq0 = (k * ncols) % JQ
                return pos_t[:, jq0 * E : (jq0 + min(ncols, JQ)) * E]

            def add_pos(g, k):
                # pos pattern repeats every JQ column blocks
                span = min(ncols, JQ) * E
                for h in range(0, ncols * E, span):
                    nc.vector.tensor_add(
                        out=g[:, h : h + span],
                        in0=g[:, h : h + span],
                        in1=pos_in1(k),
                    )

            g0 = None
            if variant == "store":
                g0 = const_pool.tile([P, ncols * E], mybir.dt.float32, tag="g0")
                nc.gpsimd.dma_gather(
                    g0[:].rearrange("p (c e) -> p c e", e=E),
                    tok[:],
                    xti_t[:, 0:icols],
                    tok_per_chunk,
                    nidx_reg,
                    E,
                )
                add_pos(g0, 0)

            def body():
                for _ in range(reps):
                    for k in range(chunks):
                        if variant == "store":
                            g = g0
                        else:
                            g = work_pool.tile(
                                [P, ncols * E], mybir.dt.float32, tag="work"
                            )
                            nc.gpsimd.dma_gather(
                                g[:].rearrange("p (c e) -> p c e", e=E),
                                tok[:],
                                xti_t[:, k * icols : (k + 1) * icols],
                                tok_per_chunk,
                                nidx_reg,
                                E,
                                single_packet=single_packet,
                                queue_num=k % nqueues,
                            )
                        if variant == "full":
                            add_pos(g, k)
                        if variant in ("full", "store", "noadd"):
                            if out_part_major:
                                out_ap = out[:, k * ncols : (k + 1) * ncols, :]
                            else:
                                out_ap = out[
                                    k * ncols : (k + 1) * ncols
                                ].rearrange("c p e -> p c e")
                            st_eng = (
                                nc.scalar if (store_alt and k % 2) else nc.sync
                            )
                            st_eng.dma_start(
                                out=out_ap,
                                in_=g[:].rearrange("p (c e) -> p c e", e=E),
                            )

            if outer > 1:
                with tc.For_i(0, outer):
                    body()
            else:
                body()

    # populate .instr bytes for extended-inst InstISA subclasses (the
    # library-reload pseudo); Bacc runs this in compile(), raw Bass doesn't
    from concourse.library_overlay import lower_extended_insts

    lower_extended_insts(nc)
    _split_multi_waits(nc)
    return nc


def make_in_maps(x32: np.ndarray, tokw: np.ndarray, posw: np.ndarray):
    in_maps = []
    for c in range(N_CORES):
        flat = x32[:, c * POS_PER_CORE : (c + 1) * POS_PER_CORE].reshape(-1)
        flat16 = flat.astype(np.int16)
        # idx i -> [i%16, i//16], replicated across the 8 groups of 16
        # partitions (one replica per GPSIMD Q7 core)
        wrapped = flat16.reshape(IDX_COLS, 16).T          # [16, 128]
        xti = np.ascontiguousarray(np.tile(wrapped, (8, 1)))  # [128, 128]
        pc = np.ascontiguousarray(posw[c * POS_PER_CORE : (c + 1) * POS_PER_CORE])
        in_maps.append({"xti": xti, "pos": pc, "tok": tokw})
    return in_maps


def unshard(results, part_major: bool = False) -> np.ndarray:
    full = np.empty((B, T, E), dtype=np.float32)
    for c in range(N_CORES):
        oc = results[c]["out"]
        if part_major:
            # [128, 16, 512] with token i at [i%128, i//128] -> [16, 128, 512]
            oc = oc.transpose(1, 0, 2)
        full[:, c * POS_PER_CORE : (c + 1) * POS_PER_CORE, :] = oc.reshape(
            B, JQ, P, E
        ).reshape(B, POS_PER_CORE, E)
    return full


def kernel(x: np.ndarray, tok_weight: np.ndarray, pos_weight: np.ndarray) -> np.ndarray:
    if "nc" not in _CACHE:
        _CACHE["nc"] = _build_program()
    nc = _CACHE["nc"]

    x32 = np.ascontiguousarray(np.asarray(x, dtype=np.int32))
    tokw = np.ascontiguousarray(np.asarray(tok_weight, dtype=np.float32))
    posw = np.ascontiguousarray(np.asarray(pos_weight, dtype=np.float32))

    in_maps = make_in_maps(x32, tokw, posw)
    results = run_bass_kernel_spmd(nc, in_maps, core_ids=list(range(N_CORES))).results
    return unshard(results)


# revision 32
# speedup vs baseline: 1.0104x; 1.0104x over previous
"""Positional embedding lookup kernel for Trainium2 (8 NeuronCores).

Problem: out[b, t, :] = tok_weight[x[b, t], :] + pos_weight[t, :]
  x:          [4, 4096]  int32/int64 token ids in [0, 32000)
  tok_weight: [32000, 512] f32
  pos_weight: [4096, 512]  f32
  out:        [4, 4096, 512] f32

Sharding: split the 4096 positions into 8 contiguous chunks of 512; core c
handles positions [c*512, (c+1)*512) for ALL 4 batches (2048 tokens).  This
makes each core read only its 1MB slice of pos_weight (reused across the 4
batches) instead of a per-token 4MB read.

Per-core flat token order: i = 0..2047 walks (b, q) = (i//512, i%512),
i.e. flat_idx = x[:, c*512:(c+1)*512].ravel().  The gather lands token i at
SBUF partition i%128, column-block i//128, so column block col corresponds
to batch col//4, position sub-block col%4 — which aligns a whole batch's
512 tokens with the (identically laid out) pos tile for a single wide add.

The row gather uses the GPSIMD dma_gather custom op (one descriptor per
row, ~0.34ns/descriptor generation) in 4 chunks of 512 rows so gather,
add, and store pipeline; indices are int16 (vocab 32000 < 32768), packed
i -> [i%16, i//16] over 16 partitions and replicated across the 8 Q7 cores.
"""

import numpy as np

import concourse.bass as bass
import concourse.tile as tile
from concourse import library_config, mybir
from concourse.bass_utils import run_bass_kernel_spmd

B = 4
T = 4096
E = 512
VOCAB = 32000
N_CORES = 8
POS_PER_CORE = T // N_CORES          # 512
TOK_PER_CORE = B * POS_PER_CORE      # 2048
P = 128
N_TILES = TOK_PER_CORE // P          # 16 column blocks of 128 tokens
JQ = POS_PER_CORE // P               # 4 pos sub-blocks
CHUNKS = 4                           # one gather/add/store chunk per batch
TOK_PER_CHUNK = TOK_PER_CORE // CHUNKS   # 512
IDX_COLS = TOK_PER_CORE // 16        # 128 int16 idx columns

_CACHE = {}


def _split_multi_waits(nc: bass.Bass) -> None:
    """Walrus codegen allows one sync-wait slot per TPB instruction (the
    NEURON_ISA_TPB_EVENTS struct); Tile can emit several.  Move extra waits
    onto standalone NoOps on the same engine, just before the instruction."""
    for func in nc.m.functions:
        for blk in func.blocks:
            new_insts = []
            for inst in blk.instructions:
                si = inst.sync_info
                if si is not None and len(si.on_wait) > 1:
                    for w in si.on_wait[:-1]:
                        nop = mybir.InstNoOp(
                            name=nc.get_next_instruction_name(),
                            engine=inst.engine,
                            bass_nofuse=True,
                            sync_info=mybir.SyncInfo(on_wait=[w], on_update=[]),
                        )
                        nc.register_instruction(nop)
                        new_insts.append(nop)
                    inst.sync_info = mybir.SyncInfo(
                        on_wait=si.on_wait[-1:], on_update=si.on_update
                    )
                new_insts.append(inst)
            blk.instructions[:] = new_insts


def _build_program(
    reps: int = 1,
    outer: int = 1,
    variant: str = "full",
    nqueues: int = 2,
    single_packet: bool = True,
    chunks: int = CHUNKS,
    out_part_major: bool = True,
    store_alt: bool = False,
    bufs: int = 3,
) -> bass.Bass:
    """reps>1 unrolls the steady-state gather/add/store loop; outer>1 wraps
    it in a runtime For_i loop.  Used for timing: the wall-time delta
    between two total rep counts isolates device time.  variant isolates
    pipeline stages for benching: "full" | "gather" | "store"."""
    nc = bass.Bass(num_swdge_queues=nqueues)

    xti = nc.declare_dram_parameter(
        "xti", [P, IDX_COLS], mybir.dt.int16, isOutput=False
    )
    pos = nc.declare_dram_parameter(
        "pos", [POS_PER_CORE, E], mybir.dt.float32, isOutput=False
    )
    tok = nc.declare_dram_parameter(
        "tok", [VOCAB, E], mybir.dt.float32, isOutput=False
    )
    out_shape = [P, N_TILES, E] if out_part_major else [N_TILES, P, E]
    out = nc.declare_dram_parameter("out", out_shape, mybir.dt.float32, isOutput=True)

    with tile.TileContext(nc) as tc:
        with (
            tc.tile_pool(name="const", bufs=1) as const_pool,
            tc.tile_pool(name="work", bufs=bufs) as work_pool,
        ):
            # dma_gather lives in the 'mlp' GPSIMD firmware library
            nc.gpsimd.load_library(library_config.mlp)

            xti_t = const_pool.tile([P, IDX_COLS], mybir.dt.int16)
            nc.sync.dma_start(out=xti_t[:], in_=xti[:])

            # all 4 pos sub-blocks in one DMA: partition p, cols
            # [jq*E:(jq+1)*E] hold pos[jq*128 + p, :]
            pos_t = const_pool.tile([P, JQ * E], mybir.dt.float32)
            nc.sync.dma_start(
                out=pos_t[:].rearrange("p (jq e) -> p jq e", jq=JQ),
                in_=pos.rearrange("(jq p) e -> p jq e", p=P),
            )
            # tiny DVE op so the vector engine observes the const-load DMA
            # semaphores once; later adds then need only the gather wait.
            obs = const_pool.tile([P, 1], mybir.dt.float32, tag="obs")
            nc.vector.tensor_copy(out=obs[:], in_=pos_t[:, 0:1])

            assert chunks in (2, 4, 8, 16)
            ncols = N_TILES // chunks            # column blocks per chunk
            icols = IDX_COLS // chunks           # idx columns per chunk
            tok_per_chunk = TOK_PER_CORE // chunks
            nidx_reg = nc.gpsimd.to_reg(tok_per_chunk)

            def pos_in1(k):
                jq0 = (k * ncols) % JQ
                return pos_t[:, jq0 * E : (jq0 + min(ncols, JQ)) * E]

            def add_pos(g, k):
                # pos pattern repeats every JQ column blocks
                span = min(ncols, JQ) * E
                for h in range(0, ncols * E, span):
                    nc.vector.tensor_add(
                        out=g[:, h : h + span],
                        in0=g[:, h : h + span],
                        in1=pos_in1(k),
                    )

            g0 = None
            if variant == "store":
                g0 = const_pool.tile([P, ncols * E], mybir.dt.float32, tag="g0")
                nc.gpsimd.dma_gather(
                    g0[:].rearrange("p (c e) -> p c e", e=E),
                    tok[:],
                    xti_t[:, 0:icols],
                    tok_per_chunk,
                    nidx_reg,
                    E,
                )
                add_pos(g0, 0)

            def body():
                for _ in range(reps):
                    for k in range(chunks):
                        if variant == "store":
                            g = g0
                        else:
                            g = work_pool.tile(
                                [P, ncols * E], mybir.dt.float32, tag="work"
                            )
                            nc.gpsimd.dma_gather(
                                g[:].rearrange("p (c e) -> p c e", e=E),
                                tok[:],
                                xti_t[:, k * icols : (k + 1) * icols],
                                tok_per_chunk,
                                nidx_reg,
                                E,
                                single_packet=single_packet,
                                queue_num=k % nqueues,
                            )
                        if variant == "full":
                            add_pos(g, k)
                        if variant in ("full", "store", "noadd"):
                            if out_part_major:
                                out_ap = out[:, k * ncols : (k + 1) * ncols, :]
                            else:
                                out_ap = out[
                                    k * ncols : (k + 1) * ncols
                                ].rearrange("c p e -> p c e")
                            st_eng = (
                                nc.scalar if (store_alt and k % 2) else nc.sync
                            )
                            st_eng.dma_start(
                                out=out_ap,
                                in_=g[:].rearrange("p (c e) -> p c e", e=E),
                            )

            if outer > 1:
                with tc.For_i(0, outer):
                    body()
            else:
                body()

    # populate .instr bytes for extended-inst InstISA subclasses (the
    # library-reload pseudo); Bacc runs this in compile(), raw Bass doesn't
    from concourse.library_overlay import lower_extended_insts

    lower_extended_insts(nc)
    _split_multi_waits(nc)
    return nc


def make_in_maps(x32: np.ndarray, tokw: np.ndarray, posw: np.ndarray):
    in_maps = []
    for c in range(N_CORES):
        flat = x32[:, c * POS_PER_CORE : (c + 1) * POS_PER_CORE].reshape(-1)
        flat16 = flat.astype(np.int16)
        # idx i -> [i%16, i//16], replicated across the 8 groups of 16
        # partitions (one replica per GPSIMD Q7 core)
        wrapped = flat16.reshape(IDX_COLS, 16).T          # [16, 128]
        xti = np.ascontiguousarray(np.tile(wrapped, (8, 1)))  # [128, 128]
        pc = np.ascontiguousarray(posw[c * POS_PER_CORE : (c + 1) * POS_PER_CORE])
        in_maps.append({"xti": xti, "pos": pc, "tok": tokw})
    return in_maps


def unshard(results, part_major: bool = False) -> np.ndarray:
    full = np.empty((B, T, E), dtype=np.float32)
    for c in range(N_CORES):
        oc = results[c]["out"]
        if part_major:
            # [128, 16, 512] with token i at [i%128, i//128] -> [16, 128, 512]
            oc = oc.transpose(1, 0, 2)
        full[:, c * POS_PER_CORE : (c + 1) * POS_PER_CORE, :] = oc.reshape(
            B, JQ, P, E
        ).reshape(B, POS_PER_CORE, E)
    return full


def kernel(x: np.ndarray, tok_weight: np.ndarray, pos_weight: np.ndarray) -> np.ndarray:
    if "nc" not in _CACHE:
        _CACHE["nc"] = _build_program()
    nc = _CACHE["nc"]

    x32 = np.ascontiguousarray(np.asarray(x, dtype=np.int32))
    tokw = np.ascontiguousarray(np.asarray(tok_weight, dtype=np.float32))
    posw = np.ascontiguousarray(np.asarray(pos_weight, dtype=np.float32))

    in_maps = make_in_maps(x32, tokw, posw)
    results = run_bass_kernel_spmd(nc, in_maps, core_ids=list(range(N_CORES))).results
    return unshard(results, part_major=True)


# revision 36
# speedup vs baseline: 1.0933x; 1.0821x over previous
"""Positional embedding lookup kernel for Trainium2 (8 NeuronCores).

Problem: out[b, t, :] = tok_weight[x[b, t], :] + pos_weight[t, :]
  x:          [4, 4096]  int32/int64 token ids in [0, 32000)
  tok_weight: [32000, 512] f32
  pos_weight: [4096, 512]  f32
  out:        [4, 4096, 512] f32

Sharding: split the 4096 positions into 8 contiguous chunks of 512; core c
handles positions [c*512, (c+1)*512) for ALL 4 batches (2048 tokens).  This
makes each core read only its 1MB slice of pos_weight (reused across the 4
batches) instead of a per-token 4MB read.

Per-core flat token order: i = 0..2047 walks (b, q) = (i//512, i%512),
i.e. flat_idx = x[:, c*512:(c+1)*512].ravel().  The gather lands token i at
SBUF partition i%128, column-block i//128, so column block col corresponds
to batch col//4, position sub-block col%4 — which aligns a whole batch's
512 tokens with the (identically laid out) pos tile for a single wide add.

The row gather uses the GPSIMD dma_gather custom op (one descriptor per
row, ~0.34ns/descriptor generation) in 4 chunks of 512 rows so gather,
add, and store pipeline; indices are int16 (vocab 32000 < 32768), packed
i -> [i%16, i//16] over 16 partitions and replicated across the 8 Q7 cores.
"""

import numpy as np

import concourse.bass as bass
import concourse.tile as tile
from concourse import library_config, mybir
from concourse.bass_utils import run_bass_kernel_spmd

B = 4
T = 4096
E = 512
VOCAB = 32000
N_CORES = 8
POS_PER_CORE = T // N_CORES          # 512
TOK_PER_CORE = B * POS_PER_CORE      # 2048
P = 128
N_TILES = TOK_PER_CORE // P          # 16 column blocks of 128 tokens
JQ = POS_PER_CORE // P               # 4 pos sub-blocks
CHUNKS = 4                           # one gather/add/store chunk per batch
TOK_PER_CHUNK = TOK_PER_CORE // CHUNKS   # 512
IDX_COLS = TOK_PER_CORE // 16        # 128 int16 idx columns

_CACHE = {}


def _split_multi_waits(nc: bass.Bass) -> None:
    """Walrus codegen allows one sync-wait slot per TPB instruction (the
    NEURON_ISA_TPB_EVENTS struct); Tile can emit several.  Move extra waits
    onto standalone NoOps on the same engine, just before the instruction."""
    for func in nc.m.functions:
        for blk in func.blocks:
            new_insts = []
            for inst in blk.instructions:
                si = inst.sync_info
                if si is not None and len(si.on_wait) > 1:
                    for w in si.on_wait[:-1]:
                        nop = mybir.InstNoOp(
                            name=nc.get_next_instruction_name(),
                            engine=inst.engine,
                            bass_nofuse=True,
                            sync_info=mybir.SyncInfo(on_wait=[w], on_update=[]),
                        )
                        nc.register_instruction(nop)
                        new_insts.append(nop)
                    inst.sync_info = mybir.SyncInfo(
                        on_wait=si.on_wait[-1:], on_update=si.on_update
                    )
                new_insts.append(inst)
            blk.instructions[:] = new_insts


def _build_program(
    reps: int = 1,
    outer: int = 1,
    variant: str = "full",
    nqueues: int = 2,
    single_packet: bool = True,
    chunks: int = CHUNKS,
    out_part_major: bool = True,
    store_alt: bool = False,
    bufs: int = 3,
    split_gather: bool = False,
) -> bass.Bass:
    """reps>1 unrolls the steady-state gather/add/store loop; outer>1 wraps
    it in a runtime For_i loop.  Used for timing: the wall-time delta
    between two total rep counts isolates device time.  variant isolates
    pipeline stages for benching: "full" | "gather" | "store"."""
    nc = bass.Bass(num_swdge_queues=nqueues)

    xti = nc.declare_dram_parameter(
        "xti", [P, IDX_COLS], mybir.dt.int16, isOutput=False
    )
    pos = nc.declare_dram_parameter(
        "pos", [POS_PER_CORE, E], mybir.dt.float32, isOutput=False
    )
    tok = nc.declare_dram_parameter(
        "tok", [VOCAB, E], mybir.dt.float32, isOutput=False
    )
    out_shape = [P, N_TILES, E] if out_part_major else [N_TILES, P, E]
    out = nc.declare_dram_parameter("out", out_shape, mybir.dt.float32, isOutput=True)

    with tile.TileContext(nc) as tc:
        with (
            tc.tile_pool(name="const", bufs=1) as const_pool,
            tc.tile_pool(name="work", bufs=bufs) as work_pool,
        ):
            # dma_gather lives in the 'mlp' GPSIMD firmware library
            nc.gpsimd.load_library(library_config.mlp)

            xti_t = const_pool.tile([P, IDX_COLS], mybir.dt.int16)
            nc.sync.dma_start(out=xti_t[:], in_=xti[:])

            # all 4 pos sub-blocks in one DMA: partition p, cols
            # [jq*E:(jq+1)*E] hold pos[jq*128 + p, :]
            pos_t = const_pool.tile([P, JQ * E], mybir.dt.float32)
            nc.sync.dma_start(
                out=pos_t[:].rearrange("p (jq e) -> p jq e", jq=JQ),
                in_=pos.rearrange("(jq p) e -> p jq e", p=P),
            )
            # tiny DVE op so the vector engine observes the const-load DMA
            # semaphores once; later adds then need only the gather wait.
            obs = const_pool.tile([P, 1], mybir.dt.float32, tag="obs")
            nc.vector.tensor_copy(out=obs[:], in_=pos_t[:, 0:1])

            assert chunks in (2, 4, 8, 16)
            ncols = N_TILES // chunks            # column blocks per chunk
            icols = IDX_COLS // chunks           # idx columns per chunk
            tok_per_chunk = TOK_PER_CORE // chunks
            nidx_reg = nc.gpsimd.to_reg(tok_per_chunk)
            half_reg = nc.gpsimd.to_reg(tok_per_chunk // 2) if split_gather else None

            def pos_in1(k):
                jq0 = (k * ncols) % JQ
                return pos_t[:, jq0 * E : (jq0 + min(ncols, JQ)) * E]

            def add_pos(g, k):
                # pos pattern repeats every JQ column blocks
                span = min(ncols, JQ) * E
                for h in range(0, ncols * E, span):
                    nc.vector.tensor_add(
                        out=g[:, h : h + span],
                        in0=g[:, h : h + span],
                        in1=pos_in1(k),
                    )

            g0 = None
            if variant == "store":
                g0 = const_pool.tile([P, ncols * E], mybir.dt.float32, tag="g0")
                nc.gpsimd.dma_gather(
                    g0[:].rearrange("p (c e) -> p c e", e=E),
                    tok[:],
                    xti_t[:, 0:icols],
                    tok_per_chunk,
                    nidx_reg,
                    E,
                )
                add_pos(g0, 0)

            def body():
                for _ in range(reps):
                    for k in range(chunks):
                        if variant == "store":
                            g = g0
                        else:
                            g = work_pool.tile(
                                [P, ncols * E], mybir.dt.float32, tag="work"
                            )
                            if split_gather:
                                hc, hi = ncols // 2 * E, icols // 2
                                hreg = half_reg
                                for h in range(2):
                                    nc.gpsimd.dma_gather(
                                        g[:, h * hc : (h + 1) * hc].rearrange(
                                            "p (c e) -> p c e", e=E
                                        ),
                                        tok[:],
                                        xti_t[
                                            :,
                                            k * icols + h * hi : k * icols
                                            + (h + 1) * hi,
                                        ],
                                        tok_per_chunk // 2,
                                        hreg,
                                        E,
                                        single_packet=single_packet,
                                        queue_num=h % nqueues,
                                    )
                            else:
                                nc.gpsimd.dma_gather(
                                    g[:].rearrange("p (c e) -> p c e", e=E),
                                    tok[:],
                                    xti_t[:, k * icols : (k + 1) * icols],
                                    tok_per_chunk,
                                    nidx_reg,
                                    E,
                                    single_packet=single_packet,
                                    queue_num=k % nqueues,
                                )
                        if variant == "full":
                            add_pos(g, k)
                        if variant in ("full", "store", "noadd"):
                            if out_part_major:
                                out_ap = out[:, k * ncols : (k + 1) * ncols, :]
                            else:
                                out_ap = out[
                                    k * ncols : (k + 1) * ncols
                                ].rearrange("c p e -> p c e")
                            st_eng = (
                                nc.scalar if (store_alt and k % 2) else nc.sync
                            )
                            st_eng.dma_start(
                                out=out_ap,
                                in_=g[:].rearrange("p (c e) -> p c e", e=E),
                            )

            if outer > 1:
                with tc.For_i(0, outer):
                    body()
            else:
                body()

    # populate .instr bytes for extended-inst InstISA subclasses (the
    # library-reload pseudo); Bacc runs this in compile(), raw Bass doesn't
    from concourse.library_overlay import lower_extended_insts

    lower_extended_insts(nc)
    _split_multi_waits(nc)
    return nc


def make_in_maps(x32: np.ndarray, tokw: np.ndarray, posw: np.ndarray):
    in_maps = []
    for c in range(N_CORES):
        flat = x32[:, c * POS_PER_CORE : (c + 1) * POS_PER_CORE].reshape(-1)
        flat16 = flat.astype(np.int16)
        # idx i -> [i%16, i//16], replicated across the 8 groups of 16
        # partitions (one replica per GPSIMD Q7 core)
        wrapped = flat16.reshape(IDX_COLS, 16).T          # [16, 128]
        xti = np.ascontiguousarray(np.tile(wrapped, (8, 1)))  # [128, 128]
        pc = np.ascontiguousarray(posw[c * POS_PER_CORE : (c + 1) * POS_PER_CORE])
        in_maps.append({"xti": xti, "pos": pc, "tok": tokw})
    return in_maps


def unshard(results, part_major: bool = False) -> np.ndarray:
    full = np.empty((B, T, E), dtype=np.float32)
    for c in range(N_CORES):
        oc = results[c]["out"]
        if part_major:
            # [128, 16, 512] with token i at [i%128, i//128] -> [16, 128, 512]
            oc = oc.transpose(1, 0, 2)
        full[:, c * POS_PER_CORE : (c + 1) * POS_PER_CORE, :] = oc.reshape(
            B, JQ, P, E
        ).reshape(B, POS_PER_CORE, E)
    return full


def kernel(x: np.ndarray, tok_weight: np.ndarray, pos_weight: np.ndarray) -> np.ndarray:
    if "nc" not in _CACHE:
        _CACHE["nc"] = _build_program()
    nc = _CACHE["nc"]

    x32 = np.ascontiguousarray(np.asarray(x, dtype=np.int32))
    tokw = np.ascontiguousarray(np.asarray(tok_weight, dtype=np.float32))
    posw = np.ascontiguousarray(np.asarray(pos_weight, dtype=np.float32))

    in_maps = make_in_maps(x32, tokw, posw)
    results = run_bass_kernel_spmd(nc, in_maps, core_ids=list(range(N_CORES))).results
    return unshard(results, part_major=True)


# revision 46
# speedup vs baseline: 1.0979x; 1.0041x over previous
"""Positional embedding lookup kernel for Trainium2 (8 NeuronCores).

Problem: out[b, t, :] = tok_weight[x[b, t], :] + pos_weight[t, :]
  x:          [4, 4096]  int32/int64 token ids in [0, 32000)
  tok_weight: [32000, 512] f32
  pos_weight: [4096, 512]  f32
  out:        [4, 4096, 512] f32

Sharding: split the 4096 positions into 8 contiguous chunks of 512; core c
handles positions [c*512, (c+1)*512) for ALL 4 batches (2048 tokens).  This
makes each core read only its 1MB slice of pos_weight (reused across the 4
batches) instead of a per-token 4MB read.

Per-core flat token order: i = 0..2047 walks (b, q) = (i//512, i%512),
i.e. flat_idx = x[:, c*512:(c+1)*512].ravel().  The gather lands token i at
SBUF partition i%128, column-block i//128, so column block col corresponds
to batch col//4, position sub-block col%4 — which aligns a whole batch's
512 tokens with the (identically laid out) pos tile for a single wide add.

The row gather uses the GPSIMD dma_gather custom op (one descriptor per
row, ~0.34ns/descriptor generation) in 4 chunks of 512 rows so gather,
add, and store pipeline; indices are int16 (vocab 32000 < 32768), packed
i -> [i%16, i//16] over 16 partitions and replicated across the 8 Q7 cores.
"""

import numpy as np

import concourse.bass as bass
import concourse.tile as tile
from concourse import library_config, mybir
from concourse.bass_utils import run_bass_kernel_spmd

B = 4
T = 4096
E = 512
VOCAB = 32000
N_CORES = 8
POS_PER_CORE = T // N_CORES          # 512
TOK_PER_CORE = B * POS_PER_CORE      # 2048
P = 128
N_TILES = TOK_PER_CORE // P          # 16 column blocks of 128 tokens
JQ = POS_PER_CORE // P               # 4 pos sub-blocks
CHUNKS = 4                           # one gather/add/store chunk per batch
TOK_PER_CHUNK = TOK_PER_CORE // CHUNKS   # 512
IDX_COLS = TOK_PER_CORE // 16        # 128 int16 idx columns
SORTED_MODE = False                  # host-sorted gather rows (see make_in_maps)

_CACHE = {}


def _split_multi_waits(nc: bass.Bass) -> None:
    """Walrus codegen allows one sync-wait slot per TPB instruction (the
    NEURON_ISA_TPB_EVENTS struct); Tile can emit several.  Move extra waits
    onto standalone NoOps on the same engine, just before the instruction."""
    for func in nc.m.functions:
        for blk in func.blocks:
            new_insts = []
            for inst in blk.instructions:
                si = inst.sync_info
                if si is not None and len(si.on_wait) > 1:
                    for w in si.on_wait[:-1]:
                        nop = mybir.InstNoOp(
                            name=nc.get_next_instruction_name(),
                            engine=inst.engine,
                            bass_nofuse=True,
                            sync_info=mybir.SyncInfo(on_wait=[w], on_update=[]),
                        )
                        nc.register_instruction(nop)
                        new_insts.append(nop)
                    inst.sync_info = mybir.SyncInfo(
                        on_wait=si.on_wait[-1:], on_update=si.on_update
                    )
                new_insts.append(inst)
            blk.instructions[:] = new_insts


def _build_program(
    reps: int = 1,
    outer: int = 1,
    variant: str = "full",
    nqueues: int = 2,
    single_packet: bool = True,
    chunks: int = CHUNKS,
    out_part_major: bool = True,
    store_alt: bool = False,
    bufs: int = 3,
    split_gather: bool = True,
    sorted_mode: bool = False,
) -> bass.Bass:
    """reps>1 unrolls the steady-state gather/add/store loop; outer>1 wraps
    it in a runtime For_i loop.  Used for timing: the wall-time delta
    between two total rep counts isolates device time.  variant isolates
    pipeline stages for benching: "full" | "gather" | "store"."""
    nc = bass.Bass(num_swdge_queues=nqueues)

    xti = nc.declare_dram_parameter(
        "xti", [P, IDX_COLS], mybir.dt.int16, isOutput=False
    )
    # sorted_mode: pos is pre-permuted per token slot (2048 rows); else the
    # core's 512 shared position rows
    pos_rows = TOK_PER_CORE if sorted_mode else POS_PER_CORE
    pos = nc.declare_dram_parameter(
        "pos", [pos_rows, E], mybir.dt.float32, isOutput=False
    )
    tok = nc.declare_dram_parameter(
        "tok", [VOCAB, E], mybir.dt.float32, isOutput=False
    )
    out_shape = [P, N_TILES, E] if out_part_major else [N_TILES, P, E]
    out = nc.declare_dram_parameter("out", out_shape, mybir.dt.float32, isOutput=True)

    with tile.TileContext(nc) as tc:
        with (
            tc.tile_pool(name="const", bufs=1) as const_pool,
            tc.tile_pool(name="work", bufs=bufs) as work_pool,
        ):
            # dma_gather lives in the 'mlp' GPSIMD firmware library
            nc.gpsimd.load_library(library_config.mlp)

            xti_t = const_pool.tile([P, IDX_COLS], mybir.dt.int16)
            nc.sync.dma_start(out=xti_t[:], in_=xti[:])

            # one DMA: partition p, col block c holds pos[c*128 + p, :]
            pos_blocks = pos_rows // P
            pos_t = const_pool.tile([P, pos_blocks * E], mybir.dt.float32)
            nc.sync.dma_start(
                out=pos_t[:].rearrange("p (c e) -> p c e", c=pos_blocks),
                in_=pos.rearrange("(c p) e -> p c e", p=P),
            )
            # tiny DVE op so the vector engine observes the const-load DMA
            # semaphores once; later adds then need only the gather wait.
            obs = const_pool.tile([P, 1], mybir.dt.float32, tag="obs")
            nc.vector.tensor_copy(out=obs[:], in_=pos_t[:, 0:1])

            assert chunks in (2, 4, 8, 16)
            ncols = N_TILES // chunks            # column blocks per chunk
            icols = IDX_COLS // chunks           # idx columns per chunk
            tok_per_chunk = TOK_PER_CORE // chunks
            nidx_reg = nc.gpsimd.to_reg(tok_per_chunk)
            half_reg = nc.gpsimd.to_reg(tok_per_chunk // 2) if split_gather else None

            def add_pos(g, k):
                if sorted_mode:
                    # pos_t is slot-aligned: one add per chunk
                    nc.vector.tensor_add(
                        out=g[:],
                        in0=g[:],
                        in1=pos_t[:, k * ncols * E : (k + 1) * ncols * E],
                    )
                    return
                # pos pattern repeats every JQ column blocks
                span = min(ncols, JQ) * E
                jq0 = (k * ncols) % JQ
                in1 = pos_t[:, jq0 * E : (jq0 + min(ncols, JQ)) * E]
                for h in range(0, ncols * E, span):
                    nc.vector.tensor_add(
                        out=g[:, h : h + span], in0=g[:, h : h + span], in1=in1
                    )

            g0 = None
            if variant == "store":
                g0 = const_pool.tile([P, ncols * E], mybir.dt.float32, tag="g0")
                nc.gpsimd.dma_gather(
                    g0[:].rearrange("p (c e) -> p c e", e=E),
                    tok[:],
                    xti_t[:, 0:icols],
                    tok_per_chunk,
                    nidx_reg,
                    E,
                )
                add_pos(g0, 0)

            def body():
                for _ in range(reps):
                    for k in range(chunks):
                        if variant == "store":
                            g = g0
                        else:
                            g = work_pool.tile(
                                [P, ncols * E], mybir.dt.float32, tag="work"
                            )
                            if split_gather:
                                hc, hi = ncols // 2 * E, icols // 2
                                hreg = half_reg
                                for h in range(2):
                                    nc.gpsimd.dma_gather(
                                        g[:, h * hc : (h + 1) * hc].rearrange(
                                            "p (c e) -> p c e", e=E
                                        ),
                                        tok[:],
                                        xti_t[
                                            :,
                                            k * icols + h * hi : k * icols
                                            + (h + 1) * hi,
                                        ],
                                        tok_per_chunk // 2,
                                        hreg,
                                        E,
                                        single_packet=single_packet,
                                        queue_num=h % nqueues,
                                    )
                            else:
                                nc.gpsimd.dma_gather(
                                    g[:].rearrange("p (c e) -> p c e", e=E),
                                    tok[:],
                                    xti_t[:, k * icols : (k + 1) * icols],
                                    tok_per_chunk,
                                    nidx_reg,
                                    E,
                                    single_packet=single_packet,
                                    queue_num=k % nqueues,
                                )
                        if variant == "full":
                            add_pos(g, k)
                        if variant in ("full", "store", "noadd"):
                            if out_part_major:
                                out_ap = out[:, k * ncols : (k + 1) * ncols, :]
                            else:
                                out_ap = out[
                                    k * ncols : (k + 1) * ncols
                                ].rearrange("c p e -> p c e")
                            st_eng = (
                                nc.scalar if (store_alt and k % 2) else nc.sync
                            )
                            st_eng.dma_start(
                                out=out_ap,
                                in_=g[:].rearrange("p (c e) -> p c e", e=E),
                            )

            if outer > 1:
                with tc.For_i(0, outer):
                    body()
            else:
                body()

    # populate .instr bytes for extended-inst InstISA subclasses (the
    # library-reload pseudo); Bacc runs this in compile(), raw Bass doesn't
    from concourse.library_overlay import lower_extended_insts

    lower_extended_insts(nc)
    _split_multi_waits(nc)
    return nc


def make_in_maps(
    x32: np.ndarray, tokw: np.ndarray, posw: np.ndarray, sorted_mode: bool = False
):
    """Returns (in_maps, orders).  sorted_mode: slot i gathers the core's
    order[i]-th token (ascending row ids, better HBM locality); pos is
    pre-permuted to stay slot-aligned and unshard inverse-permutes."""
    in_maps, orders = [], []
    for c in range(N_CORES):
        flat = x32[:, c * POS_PER_CORE : (c + 1) * POS_PER_CORE].reshape(-1)
        if sorted_mode:
            order = np.argsort(flat, kind="stable")
            vals = flat[order]
            pc = posw[c * POS_PER_CORE + (order % POS_PER_CORE)]
        else:
            order = None
            vals = flat
            pc = posw[c * POS_PER_CORE : (c + 1) * POS_PER_CORE]
        flat16 = vals.astype(np.int16)
        # idx i -> [i%16, i//16], replicated across the 8 groups of 16
        # partitions (one replica per GPSIMD Q7 core)
        wrapped = flat16.reshape(IDX_COLS, 16).T          # [16, 128]
        xti = np.ascontiguousarray(np.tile(wrapped, (8, 1)))  # [128, 128]
        in_maps.append(
            {"xti": xti, "pos": np.ascontiguousarray(pc), "tok": tokw}
        )
        orders.append(order)
    return in_maps, orders


def unshard(results, part_major: bool = False, orders=None) -> np.ndarray:
    full = np.empty((B, T, E), dtype=np.float32)
    for c in range(N_CORES):
        oc = results[c]["out"]
        if part_major:
            # [128, 16, 512] with slot i at [i%128, i//128] -> [16, 128, 512]
            oc = oc.transpose(1, 0, 2)
        rows = oc.reshape(TOK_PER_CORE, E)
        if orders is not None and orders[c] is not None:
            # slot i holds token orders[c][i]; invert the permutation
            tok_rows = np.empty_like(rows)
            tok_rows[orders[c]] = rows
            rows = tok_rows
        full[:, c * POS_PER_CORE : (c + 1) * POS_PER_CORE, :] = rows.reshape(
            B, POS_PER_CORE, E
        )
    return full


def kernel(x: np.ndarray, tok_weight: np.ndarray, pos_weight: np.ndarray) -> np.ndarray:
    if "nc" not in _CACHE:
        _CACHE["nc"] = _build_program(sorted_mode=SORTED_MODE)
    nc = _CACHE["nc"]

    x32 = np.ascontiguousarray(np.asarray(x, dtype=np.int32))
    tokw = np.ascontiguousarray(np.asarray(tok_weight, dtype=np.float32))
    posw = np.ascontiguousarray(np.asarray(pos_weight, dtype=np.float32))

    in_maps, orders = make_in_maps(x32, tokw, posw, sorted_mode=SORTED_MODE)
    results = run_bass_kernel_spmd(nc, in_maps, core_ids=list(range(N_CORES))).results
    return unshard(results, part_major=True, orders=orders)


# revision 48
# speedup vs baseline: 1.1461x; 1.0440x over previous
"""Positional embedding lookup kernel for Trainium2 (8 NeuronCores).

Problem: out[b, t, :] = tok_weight[x[b, t], :] + pos_weight[t, :]
  x:          [4, 4096]  int32/int64 token ids in [0, 32000)
  tok_weight: [32000, 512] f32
  pos_weight: [4096, 512]  f32
  out:        [4, 4096, 512] f32

Sharding: split the 4096 positions into 8 contiguous chunks of 512; core c
handles positions [c*512, (c+1)*512) for ALL 4 batches (2048 tokens).  This
makes each core read only its 1MB slice of pos_weight (reused across the 4
batches) instead of a per-token 4MB read.

Per-core flat token order: i = 0..2047 walks (b, q) = (i//512, i%512),
i.e. flat_idx = x[:, c*512:(c+1)*512].ravel().  The gather lands token i at
SBUF partition i%128, column-block i//128, so column block col corresponds
to batch col//4, position sub-block col%4 — which aligns a whole batch's
512 tokens with the (identically laid out) pos tile for a single wide add.

The row gather uses the GPSIMD dma_gather custom op (one descriptor per
row, ~0.34ns/descriptor generation) in 4 chunks of 512 rows so gather,
add, and store pipeline; indices are int16 (vocab 32000 < 32768), packed
i -> [i%16, i//16] over 16 partitions and replicated across the 8 Q7 cores.
"""

import numpy as np

import concourse.bass as bass
import concourse.tile as tile
from concourse import library_config, mybir
from concourse.bass_utils import run_bass_kernel_spmd

B = 4
T = 4096
E = 512
VOCAB = 32000
N_CORES = 8
POS_PER_CORE = T // N_CORES          # 512
TOK_PER_CORE = B * POS_PER_CORE      # 2048
P = 128
N_TILES = TOK_PER_CORE // P          # 16 column blocks of 128 tokens
JQ = POS_PER_CORE // P               # 4 pos sub-blocks
CHUNKS = 4                           # one gather/add/store chunk per batch
TOK_PER_CHUNK = TOK_PER_CORE // CHUNKS   # 512
IDX_COLS = TOK_PER_CORE // 16        # 128 int16 idx columns
SORTED_MODE = False                  # host-sorted gather rows (see make_in_maps)

_CACHE = {}


def _split_multi_waits(nc: bass.Bass) -> None:
    """Walrus codegen allows one sync-wait slot per TPB instruction (the
    NEURON_ISA_TPB_EVENTS struct); Tile can emit several.  Move extra waits
    onto standalone NoOps on the same engine, just before the instruction."""
    for func in nc.m.functions:
        for blk in func.blocks:
            new_insts = []
            for inst in blk.instructions:
                si = inst.sync_info
                if si is not None and len(si.on_wait) > 1:
                    for w in si.on_wait[:-1]:
                        nop = mybir.InstNoOp(
                            name=nc.get_next_instruction_name(),
                            engine=inst.engine,
                            bass_nofuse=True,
                            sync_info=mybir.SyncInfo(on_wait=[w], on_update=[]),
                        )
                        nc.register_instruction(nop)
                        new_insts.append(nop)
                    inst.sync_info = mybir.SyncInfo(
                        on_wait=si.on_wait[-1:], on_update=si.on_update
                    )
                new_insts.append(inst)
            blk.instructions[:] = new_insts


def _build_program(
    reps: int = 1,
    outer: int = 1,
    variant: str = "full",
    nqueues: int = 2,
    single_packet: bool = True,
    chunks: int = CHUNKS,
    out_part_major: bool = True,
    store_alt: bool = False,
    bufs: int = 3,
    split_gather: bool = True,
    sorted_mode: bool = False,
) -> bass.Bass:
    """reps>1 unrolls the steady-state gather/add/store loop; outer>1 wraps
    it in a runtime For_i loop.  Used for timing: the wall-time delta
    between two total rep counts isolates device time.  variant isolates
    pipeline stages for benching: "full" | "gather" | "store"."""
    nc = bass.Bass(num_swdge_queues=nqueues)

    xti = nc.declare_dram_parameter(
        "xti", [P, IDX_COLS], mybir.dt.int16, isOutput=False
    )
    # sorted_mode: pos is pre-permuted per token slot (2048 rows); else the
    # core's 512 shared position rows
    pos_rows = TOK_PER_CORE if sorted_mode else POS_PER_CORE
    pos = nc.declare_dram_parameter(
        "pos", [pos_rows, E], mybir.dt.float32, isOutput=False
    )
    tok = nc.declare_dram_parameter(
        "tok", [VOCAB, E], mybir.dt.float32, isOutput=False
    )
    out_shape = [P, N_TILES, E] if out_part_major else [N_TILES, P, E]
    out = nc.declare_dram_parameter("out", out_shape, mybir.dt.float32, isOutput=True)

    with tile.TileContext(nc) as tc:
        with (
            tc.tile_pool(name="const", bufs=1) as const_pool,
            tc.tile_pool(name="work", bufs=bufs) as work_pool,
        ):
            # dma_gather lives in the 'mlp' GPSIMD firmware library
            nc.gpsimd.load_library(library_config.mlp)

            xti_t = const_pool.tile([P, IDX_COLS], mybir.dt.int16)
            nc.sync.dma_start(out=xti_t[:], in_=xti[:])

            # one DMA: partition p, col block c holds pos[c*128 + p, :]
            pos_blocks = pos_rows // P
            pos_t = const_pool.tile([P, pos_blocks * E], mybir.dt.float32)
            nc.sync.dma_start(
                out=pos_t[:].rearrange("p (c e) -> p c e", c=pos_blocks),
                in_=pos.rearrange("(c p) e -> p c e", p=P),
            )
            # tiny DVE op so the vector engine observes the const-load DMA
            # semaphores once; later adds then need only the gather wait.
            obs = const_pool.tile([P, 1], mybir.dt.float32, tag="obs")
            nc.vector.tensor_copy(out=obs[:], in_=pos_t[:, 0:1])

            assert chunks in (2, 4, 8, 16)
            ncols = N_TILES // chunks            # column blocks per chunk
            icols = IDX_COLS // chunks           # idx columns per chunk
            tok_per_chunk = TOK_PER_CORE // chunks
            nidx_reg = nc.gpsimd.to_reg(tok_per_chunk)
            # split gathers into fixed 256-row pieces (the measured sweet spot)
            GATHER_ROWS = 256
            piece_reg = (
                nc.gpsimd.to_reg(GATHER_ROWS)
                if split_gather and tok_per_chunk > GATHER_ROWS
                else None
            )

            def add_pos(g, k):
                if sorted_mode:
                    # pos_t is slot-aligned: one add per chunk
                    nc.vector.tensor_add(
                        out=g[:],
                        in0=g[:],
                        in1=pos_t[:, k * ncols * E : (k + 1) * ncols * E],
                    )
                    return
                # pos pattern repeats every JQ column blocks
                span = min(ncols, JQ) * E
                jq0 = (k * ncols) % JQ
                in1 = pos_t[:, jq0 * E : (jq0 + min(ncols, JQ)) * E]
                for h in range(0, ncols * E, span):
                    nc.vector.tensor_add(
                        out=g[:, h : h + span], in0=g[:, h : h + span], in1=in1
                    )

            g0 = None
            if variant == "store":
                g0 = const_pool.tile([P, ncols * E], mybir.dt.float32, tag="g0")
                nc.gpsimd.dma_gather(
                    g0[:].rearrange("p (c e) -> p c e", e=E),
                    tok[:],
                    xti_t[:, 0:icols],
                    tok_per_chunk,
                    nidx_reg,
                    E,
                )
                add_pos(g0, 0)

            def body():
                for _ in range(reps):
                    for k in range(chunks):
                        if variant == "store":
                            g = g0
                        else:
                            g = work_pool.tile(
                                [P, ncols * E], mybir.dt.float32, tag="work"
                            )
                            if piece_reg is not None:
                                nsplit = tok_per_chunk // GATHER_ROWS
                                hc = GATHER_ROWS // P * E
                                hi = GATHER_ROWS // 16
                                for h in range(nsplit):
                                    nc.gpsimd.dma_gather(
                                        g[:, h * hc : (h + 1) * hc].rearrange(
                                            "p (c e) -> p c e", e=E
                                        ),
                                        tok[:],
                                        xti_t[
                                            :,
                                            k * icols + h * hi : k * icols
                                            + (h + 1) * hi,
                                        ],
                                        GATHER_ROWS,
                                        piece_reg,
                                        E,
                                        single_packet=single_packet,
                                        queue_num=h % nqueues,
                                    )
                            else:
                                nc.gpsimd.dma_gather(
                                    g[:].rearrange("p (c e) -> p c e", e=E),
                                    tok[:],
                                    xti_t[:, k * icols : (k + 1) * icols],
                                    tok_per_chunk,
                                    nidx_reg,
                                    E,
                                    single_packet=single_packet,
                                    queue_num=k % nqueues,
                                )
                        if variant == "full":
                            add_pos(g, k)
                        if variant in ("full", "store", "noadd"):
                            if out_part_major:
                                out_ap = out[:, k * ncols : (k + 1) * ncols, :]
                            else:
                                out_ap = out[
                                    k * ncols : (k + 1) * ncols
                                ].rearrange("c p e -> p c e")
                            st_eng = (
                                nc.scalar if (store_alt and k % 2) else nc.sync
                            )
                            st_eng.dma_start(
                                out=out_ap,
                                in_=g[:].rearrange("p (c e) -> p c e", e=E),
                            )

            if outer > 1:
                with tc.For_i(0, outer):
                    body()
            else:
                body()

    # populate .instr bytes for extended-inst InstISA subclasses (the
    # library-reload pseudo); Bacc runs this in compile(), raw Bass doesn't
    from concourse.library_overlay import lower_extended_insts

    lower_extended_insts(nc)
    _split_multi_waits(nc)
    return nc


def make_in_maps(
    x32: np.ndarray, tokw: np.ndarray, posw: np.ndarray, sorted_mode: bool = False
):
    """Returns (in_maps, orders).  sorted_mode: slot i gathers the core's
    order[i]-th token (ascending row ids, better HBM locality); pos is
    pre-permuted to stay slot-aligned and unshard inverse-permutes."""
    in_maps, orders = [], []
    for c in range(N_CORES):
        flat = x32[:, c * POS_PER_CORE : (c + 1) * POS_PER_CORE].reshape(-1)
        if sorted_mode:
            order = np.argsort(flat, kind="stable")
            vals = flat[order]
            pc = posw[c * POS_PER_CORE + (order % POS_PER_CORE)]
        else:
            order = None
            vals = flat
            pc = posw[c * POS_PER_CORE : (c + 1) * POS_PER_CORE]
        flat16 = vals.astype(np.int16)
        # idx i -> [i%16, i//16], replicated across the 8 groups of 16
        # partitions (one replica per GPSIMD Q7 core)
        wrapped = flat16.reshape(IDX_COLS, 16).T          # [16, 128]
        xti = np.ascontiguousarray(np.tile(wrapped, (8, 1)))  # [128, 128]
        in_maps.append(
            {"xti": xti, "pos": np.ascontiguousarray(pc), "tok": tokw}
        )
        orders.append(order)
    return in_maps, orders


def unshard(results, part_major: bool = False, orders=None) -> np.ndarray:
    full = np.empty((B, T, E), dtype=np.float32)
    for c in range(N_CORES):
        oc = results[c]["out"]
        if part_major:
            # [128, 16, 512] with slot i at [i%128, i//128] -> [16, 128, 512]
            oc = oc.transpose(1, 0, 2)
        rows = oc.reshape(TOK_PER_CORE, E)
        if orders is not None and orders[c] is not None:
            # slot i holds token orders[c][i]; invert the permutation
            tok_rows = np.empty_like(rows)
            tok_rows[orders[c]] = rows
            rows = tok_rows
        full[:, c * POS_PER_CORE : (c + 1) * POS_PER_CORE, :] = rows.reshape(
            B, POS_PER_CORE, E
        )
    return full


def kernel(x: np.ndarray, tok_weight: np.ndarray, pos_weight: np.ndarray) -> np.ndarray:
    if "nc" not in _CACHE:
        _CACHE["nc"] = _build_program(sorted_mode=SORTED_MODE)
    nc = _CACHE["nc"]

    x32 = np.ascontiguousarray(np.asarray(x, dtype=np.int32))
    tokw = np.ascontiguousarray(np.asarray(tok_weight, dtype=np.float32))
    posw = np.ascontiguousarray(np.asarray(pos_weight, dtype=np.float32))

    in_maps, orders = make_in_maps(x32, tokw, posw, sorted_mode=SORTED_MODE)
    results = run_bass_kernel_spmd(nc, in_maps, core_ids=list(range(N_CORES))).results
    return unshard(results, part_major=True, orders=orders)


# revision 49
# speedup vs baseline: 1.6450x; 1.4353x over previous
"""Positional embedding lookup kernel for Trainium2 (8 NeuronCores).

Problem: out[b, t, :] = tok_weight[x[b, t], :] + pos_weight[t, :]
  x:          [4, 4096]  int32/int64 token ids in [0, 32000)
  tok_weight: [32000, 512] f32
  pos_weight: [4096, 512]  f32
  out:        [4, 4096, 512] f32

Sharding: split the 4096 positions into 8 contiguous chunks of 512; core c
handles positions [c*512, (c+1)*512) for ALL 4 batches (2048 tokens).  This
makes each core read only its 1MB slice of pos_weight (reused across the 4
batches) instead of a per-token 4MB read.

Per-core flat token order: i = 0..2047 walks (b, q) = (i//512, i%512),
i.e. flat_idx = x[:, c*512:(c+1)*512].ravel().  The gather lands token i at
SBUF partition i%128, column-block i//128, so column block col corresponds
to batch col//4, position sub-block col%4 — which aligns a whole batch's
512 tokens with the (identically laid out) pos tile for a single wide add.

The row gather uses the GPSIMD dma_gather custom op (one descriptor per
row, ~0.34ns/descriptor generation) in 4 chunks of 512 rows so gather,
add, and store pipeline; indices are int16 (vocab 32000 < 32768), packed
i -> [i%16, i//16] over 16 partitions and replicated across the 8 Q7 cores.
"""

import numpy as np

import concourse.bass as bass
import concourse.tile as tile
from concourse import library_config, mybir
from concourse.bass_utils import run_bass_kernel_spmd

B = 4
T = 4096
E = 512
VOCAB = 32000
N_CORES = 8
POS_PER_CORE = T // N_CORES          # 512
TOK_PER_CORE = B * POS_PER_CORE      # 2048
P = 128
N_TILES = TOK_PER_CORE // P          # 16 column blocks of 128 tokens
JQ = POS_PER_CORE // P               # 4 pos sub-blocks
CHUNKS = 4                           # one gather/add/store chunk per batch
TOK_PER_CHUNK = TOK_PER_CORE // CHUNKS   # 512
IDX_COLS = TOK_PER_CORE // 16        # 128 int16 idx columns
SORTED_MODE = False                  # host-sorted gather rows (see make_in_maps)

_CACHE = {}


def _split_multi_waits(nc: bass.Bass) -> None:
    """Walrus codegen allows one sync-wait slot per TPB instruction (the
    NEURON_ISA_TPB_EVENTS struct); Tile can emit several.  Move extra waits
    onto standalone NoOps on the same engine, just before the instruction."""
    for func in nc.m.functions:
        for blk in func.blocks:
            new_insts = []
            for inst in blk.instructions:
                si = inst.sync_info
                if si is not None and len(si.on_wait) > 1:
                    for w in si.on_wait[:-1]:
                        nop = mybir.InstNoOp(
                            name=nc.get_next_instruction_name(),
                            engine=inst.engine,
                            bass_nofuse=True,
                            sync_info=mybir.SyncInfo(on_wait=[w], on_update=[]),
                        )
                        nc.register_instruction(nop)
                        new_insts.append(nop)
                    inst.sync_info = mybir.SyncInfo(
                        on_wait=si.on_wait[-1:], on_update=si.on_update
                    )
                new_insts.append(inst)
            blk.instructions[:] = new_insts


def _build_program(
    reps: int = 1,
    outer: int = 1,
    variant: str = "full",
    nqueues: int = 2,
    single_packet: bool = True,
    chunks: int = CHUNKS,
    out_part_major: bool = True,
    store_alt: bool = False,
    bufs: int = 3,
    split_gather: bool = True,
    sorted_mode: bool = False,
) -> bass.Bass:
    """reps>1 unrolls the steady-state gather/add/store loop; outer>1 wraps
    it in a runtime For_i loop.  Used for timing: the wall-time delta
    between two total rep counts isolates device time.  variant isolates
    pipeline stages for benching: "full" | "gather" | "store"."""
    nc = bass.Bass(num_swdge_queues=nqueues)

    xti = nc.declare_dram_parameter(
        "xti", [P, IDX_COLS], mybir.dt.int16, isOutput=False
    )
    # sorted_mode: pos is pre-permuted per token slot (2048 rows); else the
    # core's 512 shared position rows
    pos_rows = TOK_PER_CORE if sorted_mode else POS_PER_CORE
    pos = nc.declare_dram_parameter(
        "pos", [pos_rows, E], mybir.dt.float32, isOutput=False
    )
    tok = nc.declare_dram_parameter(
        "tok", [VOCAB, E], mybir.dt.float32, isOutput=False
    )
    out_shape = [P, N_TILES, E] if out_part_major else [N_TILES, P, E]
    out = nc.declare_dram_parameter("out", out_shape, mybir.dt.float32, isOutput=True)

    with tile.TileContext(nc) as tc:
        with (
            tc.tile_pool(name="const", bufs=1) as const_pool,
            tc.tile_pool(name="work", bufs=bufs) as work_pool,
        ):
            # dma_gather lives in the 'mlp' GPSIMD firmware library
            nc.gpsimd.load_library(library_config.mlp)

            xti_t = const_pool.tile([P, IDX_COLS], mybir.dt.int16)
            nc.sync.dma_start(out=xti_t[:], in_=xti[:])

            # one DMA: partition p, col block c holds pos[c*128 + p, :]
            pos_blocks = pos_rows // P
            pos_t = const_pool.tile([P, pos_blocks * E], mybir.dt.float32)
            nc.sync.dma_start(
                out=pos_t[:].rearrange("p (c e) -> p c e", c=pos_blocks),
                in_=pos.rearrange("(c p) e -> p c e", p=P),
            )
            # tiny DVE op so the vector engine observes the const-load DMA
            # semaphores once; later adds then need only the gather wait.
            obs = const_pool.tile([P, 1], mybir.dt.float32, tag="obs")
            nc.vector.tensor_copy(out=obs[:], in_=pos_t[:, 0:1])

            assert chunks in (1, 2, 4, 8, 16)
            ncols = N_TILES // chunks            # column blocks per chunk
            icols = IDX_COLS // chunks           # idx columns per chunk
            tok_per_chunk = TOK_PER_CORE // chunks
            nidx_reg = nc.gpsimd.to_reg(tok_per_chunk)
            # split gathers into fixed 256-row pieces (the measured sweet spot)
            GATHER_ROWS = 256
            piece_reg = (
                nc.gpsimd.to_reg(GATHER_ROWS)
                if split_gather and tok_per_chunk > GATHER_ROWS
                else None
            )

            def add_pos(g, k):
                if sorted_mode:
                    # pos_t is slot-aligned: one add per chunk
                    nc.vector.tensor_add(
                        out=g[:],
                        in0=g[:],
                        in1=pos_t[:, k * ncols * E : (k + 1) * ncols * E],
                    )
                    return
                # pos pattern repeats every JQ column blocks
                span = min(ncols, JQ) * E
                jq0 = (k * ncols) % JQ
                in1 = pos_t[:, jq0 * E : (jq0 + min(ncols, JQ)) * E]
                for h in range(0, ncols * E, span):
                    nc.vector.tensor_add(
                        out=g[:, h : h + span], in0=g[:, h : h + span], in1=in1
                    )

            g0 = None
            if variant == "store":
                g0 = const_pool.tile([P, ncols * E], mybir.dt.float32, tag="g0")
                nc.gpsimd.dma_gather(
                    g0[:].rearrange("p (c e) -> p c e", e=E),
                    tok[:],
                    xti_t[:, 0:icols],
                    tok_per_chunk,
                    nidx_reg,
                    E,
                )
                add_pos(g0, 0)

            def body():
                for _ in range(reps):
                    for k in range(chunks):
                        if variant == "store":
                            g = g0
                        else:
                            g = work_pool.tile(
                                [P, ncols * E], mybir.dt.float32, tag="work"
                            )
                            if piece_reg is not None:
                                nsplit = tok_per_chunk // GATHER_ROWS
                                hc = GATHER_ROWS // P * E
                                hi = GATHER_ROWS // 16
                                for h in range(nsplit):
                                    nc.gpsimd.dma_gather(
                                        g[:, h * hc : (h + 1) * hc].rearrange(
                                            "p (c e) -> p c e", e=E
                                        ),
                                        tok[:],
                                        xti_t[
                                            :,
                                            k * icols + h * hi : k * icols
                                            + (h + 1) * hi,
                                        ],
                                        GATHER_ROWS,
                                        piece_reg,
                                        E,
                                        single_packet=single_packet,
                                        queue_num=h % nqueues,
                                    )
                            else:
                                nc.gpsimd.dma_gather(
                                    g[:].rearrange("p (c e) -> p c e", e=E),
                                    tok[:],
                                    xti_t[:, k * icols : (k + 1) * icols],
                                    tok_per_chunk,
                                    nidx_reg,
                                    E,
                                    single_packet=single_packet,
                                    queue_num=k % nqueues,
                                )
                        if variant == "full":
                            add_pos(g, k)
                        if variant in ("full", "store", "noadd"):
                            if out_part_major:
                                out_ap = out[:, k * ncols : (k + 1) * ncols, :]
                            else:
                                out_ap = out[
                                    k * ncols : (k + 1) * ncols
                                ].rearrange("c p e -> p c e")
                            st_eng = (
                                nc.scalar if (store_alt and k % 2) else nc.sync
                            )
                            st_eng.dma_start(
                                out=out_ap,
                                in_=g[:].rearrange("p (c e) -> p c e", e=E),
                            )

            if outer > 1:
                with tc.For_i(0, outer):
                    body()
            else:
                body()

    # populate .instr bytes for extended-inst InstISA subclasses (the
    # library-reload pseudo); Bacc runs this in compile(), raw Bass doesn't
    from concourse.library_overlay import lower_extended_insts

    lower_extended_insts(nc)
    _split_multi_waits(nc)
    return nc


def make_in_maps(
    x32: np.ndarray, tokw: np.ndarray, posw: np.ndarray, sorted_mode: bool = False
):
    """Returns (in_maps, orders).  sorted_mode: slot i gathers the core's
    order[i]-th token (ascending row ids, better HBM locality); pos is
    pre-permuted to stay slot-aligned and unshard inverse-permutes."""
    in_maps, orders = [], []
    for c in range(N_CORES):
        flat = x32[:, c * POS_PER_CORE : (c + 1) * POS_PER_CORE].reshape(-1)
        if sorted_mode:
            order = np.argsort(flat, kind="stable")
            vals = flat[order]
            pc = posw[c * POS_PER_CORE + (order % POS_PER_CORE)]
        else:
            order = None
            vals = flat
            pc = posw[c * POS_PER_CORE : (c + 1) * POS_PER_CORE]
        flat16 = vals.astype(np.int16)
        # idx i -> [i%16, i//16], replicated across the 8 groups of 16
        # partitions (one replica per GPSIMD Q7 core)
        wrapped = flat16.reshape(IDX_COLS, 16).T          # [16, 128]
        xti = np.ascontiguousarray(np.tile(wrapped, (8, 1)))  # [128, 128]
        in_maps.append(
            {"xti": xti, "pos": np.ascontiguousarray(pc), "tok": tokw}
        )
        orders.append(order)
    return in_maps, orders


def unshard(results, part_major: bool = False, orders=None) -> np.ndarray:
    full = np.empty((B, T, E), dtype=np.float32)
    for c in range(N_CORES):
        oc = results[c]["out"]
        if part_major:
            # [128, 16, 512] with slot i at [i%128, i//128] -> [16, 128, 512]
            oc = oc.transpose(1, 0, 2)
        rows = oc.reshape(TOK_PER_CORE, E)
        if orders is not None and orders[c] is not None:
            # slot i holds token orders[c][i]; invert the permutation
            tok_rows = np.empty_like(rows)
            tok_rows[orders[c]] = rows
            rows = tok_rows
        full[:, c * POS_PER_CORE : (c + 1) * POS_PER_CORE, :] = rows.reshape(
            B, POS_PER_CORE, E
        )
    return full


def kernel(x: np.ndarray, tok_weight: np.ndarray, pos_weight: np.ndarray) -> np.ndarray:
    if "nc" not in _CACHE:
        _CACHE["nc"] = _build_program(sorted_mode=SORTED_MODE)
    nc = _CACHE["nc"]

    x32 = np.ascontiguousarray(np.asarray(x, dtype=np.int32))
    tokw = np.ascontiguousarray(np.asarray(tok_weight, dtype=np.float32))
    posw = np.ascontiguousarray(np.asarray(pos_weight, dtype=np.float32))

    in_maps, orders = make_in_maps(x32, tokw, posw, sorted_mode=SORTED_MODE)
    results = run_bass_kernel_spmd(nc, in_maps, core_ids=list(range(N_CORES))).results
    return unshard(results, part_major=True, orders=orders)
